# revision 61
# baseline (speedup 1.0000x reference)
"""Trainium2 Bass kernel for the show-attend-tell style attention module.

Per batch image b:
  att_enc = encoder_out[b] @ W_enc                      # [P, A]
  c_b     = decoder_hidden[b] @ W_dec + b_enc + b_dec   # [A]
  energy  = relu(att_enc + c_b) @ w_full  (+ b_full)    # [P]
  alpha   = softmax(energy)                             # [P]
  context = alpha @ encoder_out[b]                      # [E]

Data-parallel: batch 512 sharded as 64 per NeuronCore across 8 cores.
b_full cancels inside softmax, and measured energies are in [-1.6, 1.3],
so exp() runs without max subtraction.

Batches are processed in PAIRS with their pixel axes concatenated
(392 = 2*196), which halves matmul/DVE/ACT instruction counts:
  X bf16 (SWDGE cast-DMA) -> TensorE transposes -> X.T pair chunks
  [128e, 392p] -> Y.T = W_enc.T @ X.T (N=392 matmuls, weights
  stationary) -> ScalarE relu+bias per batch half -> energy row
  [1, 392] via w_full-column matmuls -> exp / softmax on the row ->
  exp columns via tiny transposes -> context matmuls col-tiled across
  the pair (PE array columns 0-31 / 32-63 concurrently).
The emit loop is software-pipelined (head(p), energy(p-1), ctx(p-2))
so PE never waits on ScalarE/VectorE evacuations.
"""

from contextlib import ExitStack

import numpy as np

import concourse.bass as bass
import concourse.tile as tile
from concourse import bacc, mybir
from concourse.bass_utils import run_bass_kernel_spmd
from concourse.masks import make_identity

N_CORES = 8
B, P, E, A = 512, 196, 512, 512
BL = B // N_CORES          # 64 batches per core
P0, P1 = 128, P - 128      # 128 + 68 pixel partition tiles
P2 = 2 * P                 # pair-fused pixel axis
EC = E // 128              # 4 contraction chunks
AC = A // 128              # 4 attention-dim chunks

F32 = mybir.dt.float32
BF16 = mybir.dt.bfloat16
AF = mybir.ActivationFunctionType


def build_kernel(BL=BL):
    nc = bacc.Bacc("TRN2", target_bir_lowering=False, debug=False)

    enc = nc.declare_dram_parameter("encoder_out", [BL, P, E], F32, isOutput=False)
    dec = nc.declare_dram_parameter("decoder_hidden", [BL, E], F32, isOutput=False)
    w_enc = nc.declare_dram_parameter("W_enc", [E, A], F32, isOutput=False)
    b_enc = nc.declare_dram_parameter("b_enc", [A], F32, isOutput=False)
    w_dec = nc.declare_dram_parameter("W_dec", [E, A], F32, isOutput=False)
    b_dec = nc.declare_dram_parameter("b_dec", [A], F32, isOutput=False)
    w_full = nc.declare_dram_parameter("w_full", [A], F32, isOutput=False)
    ctx_out = nc.declare_dram_parameter("context", [BL, E], F32, isOutput=True)
    alpha_out = nc.declare_dram_parameter("alpha", [BL, P], F32, isOutput=True)

    # input DMA groups (batches per SWDGE load): small first groups so the
    # PE can start before the bulk of the first loads land
    sizes = []
    left = BL
    for s in [2, 2, 2, 2, 4, 4]:
        if left >= s:
            sizes.append(s)
            left -= s
    while left > 0:
        s = min(8, left)
        sizes.append(s)
        left -= s
    groups = []
    start = 0
    for s in sizes:
        groups.append((start, s))
        start += s
    pair_group = {}
    for gi, (gs, gn) in enumerate(groups):
        for b in range(gs, gs + gn, 2):
            pair_group[b // 2] = (gi, (b - gs))

    with tile.TileContext(nc) as tc, ExitStack() as ctx:
        const = ctx.enter_context(tc.tile_pool(name="const", bufs=1))
        xf_pool = ctx.enter_context(tc.tile_pool(name="xf", bufs=2))
        xg_pool = ctx.enter_context(tc.tile_pool(name="xg", bufs=5))
        xt_pool = ctx.enter_context(tc.tile_pool(name="xt", bufs=3))
        rt_pool = ctx.enter_context(tc.tile_pool(name="rt", bufs=4))
        rows = ctx.enter_context(tc.tile_pool(name="rows", bufs=4))
        tp_ps = ctx.enter_context(tc.tile_pool(name="tp_ps", bufs=2, space="PSUM"))
        yp_ps = ctx.enter_context(tc.tile_pool(name="yp_ps", bufs=3, space="PSUM"))
        ep_ps = ctx.enter_context(tc.tile_pool(name="ep_ps", bufs=1, space="PSUM"))
        cp_ps = ctx.enter_context(tc.tile_pool(name="cp_ps", bufs=2, space="PSUM"))

        # ---- constants ----
        state = {}

        def emit_group_dma(gi):
            gs, gn = groups[gi]
            xg = xg_pool.tile([128, 8, 2, E], BF16, tag="xg")
            if gi < 2:
                # ramp groups ride the HWDGE/sync queue (in parallel with
                # the SWDGE weight loads) as f32 + ScalarE convert; the
                # head transposes read the f32 tile directly so only the
                # context path (2 pairs later) waits on the convert
                xf = xf_pool.tile([128, 2, 2, E], F32, tag="xf")
                nc.sync.dma_start(
                    out=xf[:, 0:gn, 0, :],
                    in_=enc[gs:gs + gn, 0:P0, :].rearrange("b p e -> p b e"))
                nc.sync.dma_start(
                    out=xf[0:P1, 0:gn, 1, :],
                    in_=enc[gs:gs + gn, P0:P, :].rearrange("b p e -> p b e"))
                nc.scalar.copy(xg[:, 0:gn, 0, :], xf[:, 0:gn, 0, :])
                nc.scalar.copy(xg[0:P1, 0:gn, 1, :], xf[0:P1, 0:gn, 1, :])
                state[("xf", gi)] = xf
            else:
                nc.gpsimd.dma_start(
                    out=xg[:, 0:gn, 0, :],
                    in_=enc[gs:gs + gn, 0:P0, :].rearrange("b p e -> p b e"))
                nc.gpsimd.dma_start(
                    out=xg[0:P1, 0:gn, 1, :],
                    in_=enc[gs:gs + gn, P0:P, :].rearrange("b p e -> p b e"))
            state[("g", gi)] = xg

        # Startup-critical SWDGE queue order (single FIFO per engine):
        # group 0's X, then W_enc chunk 0 (first main matmuls),
        # decoder_hidden + W_dec (the C.T preamble feeding the first
        # relus), group 1, remaining W_enc chunks, ...  All weights load
        # as bf16 via cast-DMA so no on-chip converts gate anything.
        wenc_bf = const.tile([128, EC, A], BF16)   # [e_in_chunk, e_chunk, a]
        wdec_bf = const.tile([128, EC, A], BF16)
        wf_bf = const.tile([128, AC], BF16)        # w_full as columns per a-chunk
        h_bf = const.tile([BL, E], BF16)

        ident = const.tile([128, 128], BF16)
        make_identity(nc, ident)
        identf = const.tile([128, 128], F32)
        make_identity(nc, identf)

        emit_group_dma(0)
        nc.gpsimd.dma_start(out=h_bf, in_=dec[:, :])
        nc.gpsimd.dma_start(out=wenc_bf[:, 0, :], in_=w_enc[0:128, :])
        for c in range(EC):
            nc.gpsimd.dma_start(out=wdec_bf[:, c, :],
                                in_=w_dec[c * 128:(c + 1) * 128, :])
        if len(groups) > 1:
            emit_group_dma(1)
        for c in range(1, EC):
            nc.gpsimd.dma_start(out=wenc_bf[:, c, :],
                                in_=w_enc[c * 128:(c + 1) * 128, :])
        nc.gpsimd.dma_start(out=wf_bf, in_=w_full.ap().rearrange("(c p) -> p c", p=128))
        if len(groups) > 2:
            emit_group_dma(2)

        bias_e = const.tile([128, AC], F32)
        bias_d = const.tile([128, AC], F32)
        bias_c = const.tile([128, AC], F32)
        nc.sync.dma_start(out=bias_e, in_=b_enc.ap().rearrange("(c p) -> p c", p=128))
        nc.sync.dma_start(out=bias_d, in_=b_dec.ap().rearrange("(c p) -> p c", p=128))
        nc.vector.tensor_add(bias_c, bias_e, bias_d)

        # HAM warm-up: the PE is otherwise idle while the first X group and
        # W_dec load, and the clock gate needs ~3.4us of matmul activity to
        # reach 2.4 GHz; burn the wait on dummy matmuls (they only depend on
        # the identity tile) so the first real pairs run warm
        warm = ep_ps.tile([128, 512], F32, tag="ep")
        for _ in range(28):
            nc.tensor.matmul(warm[:, 0:128], ident, ident, start=True, stop=True)

        # H.T [e, b] via TensorE transposes
        ht_bf = const.tile([128, EC, BL], BF16)
        for c in range(EC):
            tp = tp_ps.tile([128, P2], BF16, tag="tp")
            nc.tensor.transpose(tp[:, 0:BL], h_bf[0:BL, c * 128:(c + 1) * 128],
                                ident[0:BL, 0:BL])
            nc.vector.tensor_copy(ht_bf[:, c, :], tp[:, 0:BL])

        # C.T[a, b] = W_dec.T @ H.T + (b_enc + b_dec), per-partition bias add
        ct = const.tile([128, AC, BL], F32)
        for ac in range(AC):
            cps = tp_ps.tile([128, P2], F32, tag="tp")
            for ec in range(EC):
                nc.tensor.matmul(cps[:, 0:BL], wdec_bf[:, ec, ac * 128:(ac + 1) * 128],
                                 ht_bf[:, ec, :], start=(ec == 0), stop=(ec == EC - 1))
            nc.scalar.activation(ct[:, ac, :], cps[:, 0:BL], AF.Identity,
                                 bias=bias_c[:, ac:ac + 1])

        # ---- pair loop, software-pipelined ----
        def emit_head(pbs):
            # one or two pairs; with two, the main matmuls run ec-major
            # across both pairs so consecutive matmuls share the stationary
            # W_enc tile (bacc elides the duplicate LDWEIGHTS)
            for pb in pbs:
                gi, off = pair_group[pb]
                if ("g", gi) not in state:
                    emit_group_dma(gi)
                xg = state[("g", gi)]
                x0a = xg[:, off, 0, :]           # batch b   pixels 0:128
                x1a = xg[0:P1, off, 1, :]        # batch b   pixels 128:196
                x0b = xg[:, off + 1, 0, :]       # batch b+1 pixels 0:128
                x1b = xg[0:P1, off + 1, 1, :]    # batch b+1 pixels 128:196

                # X.T pair chunks [128e, 392p] = [b0:196 | b1:196]; ramp
                # groups transpose from the f32 tile (fp32 transpose-mode
                # is full rate, copy does the bf16 cast)
                if ("xf", gi) in state:
                    xf = state[("xf", gi)]
                    t0a, t1a = xf[:, off, 0, :], xf[0:P1, off, 1, :]
                    t0b, t1b = xf[:, off + 1, 0, :], xf[0:P1, off + 1, 1, :]
                    tid, tdt = identf, F32
                else:
                    t0a, t1a, t0b, t1b = x0a, x1a, x0b, x1b
                    tid, tdt = ident, BF16
                xt = xt_pool.tile([128, EC, P2], BF16, tag="xt")
                for ec in range(EC):
                    sl = slice(ec * 128, (ec + 1) * 128)
                    tp = tp_ps.tile([128, P2], tdt, tag="tp")
                    nc.tensor.transpose(tp[:, 0:P0], t0a[:, sl], tid)
                    nc.tensor.transpose(tp[:, P0:P], t1a[:, sl], tid[0:P1, 0:P1])
                    nc.tensor.transpose(tp[:, P:P + P0], t0b[:, sl], tid)
                    nc.tensor.transpose(tp[:, P + P0:P2], t1b[:, sl], tid[0:P1, 0:P1])
                    nc.vector.tensor_copy(xt[:, ec, :], tp)
                state[pb] = dict(x0a=x0a, x1a=x1a, x0b=x0b, x1b=x1b, xt=xt)

            # Y.T pair chunks + fused relu/bias/cast per batch half
            for pb in pbs:
                b = 2 * pb
                rt = rt_pool.tile([128, AC, P2], BF16, tag="rt")
                state[pb]["rt"] = rt
                for ac in range(AC):
                    yp = yp_ps.tile([128, P2], F32, tag="yp")
                    for ec in range(EC):
                        nc.tensor.matmul(yp, wenc_bf[:, ec, ac * 128:(ac + 1) * 128],
                                         state[pb]["xt"][:, ec, :],
                                         start=(ec == 0), stop=(ec == EC - 1))
                    nc.scalar.activation(rt[:, ac, 0:P], yp[:, 0:P], AF.Relu,
                                         bias=ct[:, ac, b:b + 1])
                    nc.scalar.activation(rt[:, ac, P:P2], yp[:, P:P2], AF.Relu,
                                         bias=ct[:, ac, b + 1:b + 2])

        def emit_energy(pairs):
            # energy matmuls for one or two pairs; with two pairs the
            # accumulation chains run in PE array column groups 0-31 and
            # 32-63 concurrently (separate PSUM banks for has_written)
            eps = []
            for k, pb in enumerate(pairs):
                ep = ep_ps.tile([64, 512], F32, tag="ep")
                eps.append(ep[32 * k:32 * k + 1, 0:P2])
            for ac in range(AC):
                for k, pb in enumerate(pairs):
                    nc.tensor.matmul(eps[k], wf_bf[:, ac:ac + 1],
                                     state[pb]["rt"][:, ac, :],
                                     start=(ac == 0), stop=(ac == AC - 1),
                                     tile_position=(0, 32 * k))
            # softmax tail (no max subtraction; energies are O(1)); the
            # second pair's row lives on partition 32 all the way through
            for k, pb in enumerate(pairs):
                b = 2 * pb
                st = state[pb]
                o = 32 * k
                exp_t = rows.tile([33, P2], BF16, tag="exp")
                zsum = rows.tile([33, 2], F32, tag="zsum")
                rz_t = rows.tile([33, 2], F32, tag="rz")
                exp_bf = exp_t[o:o + 1, :]
                zs = zsum[o:o + 1, :]
                rz = rz_t[o:o + 1, :]
                nc.scalar.activation(exp_bf, eps[k], AF.Exp)
                nc.vector.tensor_reduce(zs, exp_bf.rearrange("r (b p) -> r b p", b=2),
                                        axis=mybir.AxisListType.X, op=mybir.AluOpType.add)
                nc.vector.reciprocal(rz, zs)
                alpha_t = rows.tile([33, P2], F32, tag="alpha")
                alpha_row = alpha_t[o:o + 1, :]
                nc.scalar.mul(alpha_row[:, 0:P], exp_bf[:, 0:P], rz[:, 0:1])
                nc.scalar.mul(alpha_row[:, P:P2], exp_bf[:, P:P2], rz[:, 1:2])
                nc.sync.dma_start(out=alpha_out[b:b + 2, :], in_=alpha_row)
                st.update(exp_bf=exp_bf, rz=rz, kofs=o)

        def emit_ctx(pb):
            b = 2 * pb
            st = state.pop(pb)
            exp_bf, rz, o = st["exp_bf"], st["rz"], st["kofs"]
            idk = ident[o:o + 1, o:o + 1]
            # exp columns: psum cols {0,2} = batch b p0/p1, {4,6} = batch b+1
            ecol_p = tp_ps.tile([128, 8], BF16, tag="tp")
            nc.tensor.transpose(ecol_p[:, 0:1], exp_bf[0:1, 0:P0], idk)
            nc.tensor.transpose(ecol_p[0:P1, 2:3], exp_bf[0:1, P0:P], idk)
            nc.tensor.transpose(ecol_p[:, 4:5], exp_bf[0:1, P:P + P0], idk)
            nc.tensor.transpose(ecol_p[0:P1, 6:7], exp_bf[0:1, P + P0:P2], idk)
            ecol = rows.tile([128, 4], BF16, tag="ecolsb")
            nc.vector.tensor_copy(ecol[:, 0:2], ecol_p[:, 0:5:4])        # p0 cols b, b+1
            nc.vector.tensor_copy(ecol[0:P1, 2:4], ecol_p[0:P1, 2:7:4])  # p1 cols b, b+1

            # context matmuls, col-tiled across the pair (array cols 0-31 / 32-63)
            cpa = cp_ps.tile([64, 512], F32, tag="cp")
            cpb = cp_ps.tile([64, 512], F32, tag="cp")
            nc.tensor.matmul(cpa[0:1, :], ecol[:, 0:1], st["x0a"],
                             start=True, stop=False, tile_position=(0, 0))
            nc.tensor.matmul(cpb[32:33, :], ecol[:, 1:2], st["x0b"],
                             start=True, stop=False, tile_position=(0, 32))
            nc.tensor.matmul(cpa[0:1, :], ecol[0:P1, 2:3], st["x1a"],
                             start=False, stop=True, tile_position=(0, 0))
            nc.tensor.matmul(cpb[32:33, :], ecol[0:P1, 3:4], st["x1b"],
                             start=False, stop=True, tile_position=(0, 32))
            ctx2 = rows.tile([33, E], F32, tag="ctx")
            nc.vector.tensor_scalar_mul(ctx2[0:1, :], cpa[0:1, :], rz[:, 0:1])
            nc.vector.tensor_scalar_mul(ctx2[32:33, :], cpb[32:33, :], rz[:, 1:2])
            nc.sync.dma_start(out=ctx_out[b:b + 2, :], in_=ctx2[0:33:32, :])

        NP = BL // 2
        for pb in range(NP):
            emit_head([pb])
            if pb >= 1:
                emit_energy([pb - 1])
            if pb >= 2:
                emit_ctx(pb - 2)
        emit_energy([NP - 1])
        if NP >= 2:
            emit_ctx(NP - 2)
        emit_ctx(NP - 1)

    nc.compile()
    return nc


_NC = None


def kernel(**inputs):
    global _NC
    if _NC is None:
        _NC = build_kernel()
    nc = _NC

    enc = np.ascontiguousarray(inputs["encoder_out"], dtype=np.float32)
    dec = np.ascontiguousarray(inputs["decoder_hidden"], dtype=np.float32)
    shared = {
        "W_enc": np.ascontiguousarray(inputs["W_enc"], dtype=np.float32),
        "b_enc": np.ascontiguousarray(inputs["b_enc"], dtype=np.float32),
        "W_dec": np.ascontiguousarray(inputs["W_dec"], dtype=np.float32),
        "b_dec": np.ascontiguousarray(inputs["b_dec"], dtype=np.float32),
        "w_full": np.ascontiguousarray(inputs["w_full"], dtype=np.float32),
    }
    in_maps = []
    for i in range(N_CORES):
        m = dict(shared)
        m["encoder_out"] = enc[i * BL:(i + 1) * BL]
        m["decoder_hidden"] = dec[i * BL:(i + 1) * BL]
        in_maps.append(m)

    res = run_bass_kernel_spmd(nc, in_maps, list(range(N_CORES)))
    context = np.concatenate([res.results[i]["context"] for i in range(N_CORES)], axis=0)
    alpha = np.concatenate([res.results[i]["alpha"] for i in range(N_CORES)], axis=0)
    return context.astype(np.float32), alpha.astype(np.float32)


# revision 65
# speedup vs baseline: 1.0179x; 1.0179x over previous
"""Trainium2 Bass kernel for the show-attend-tell style attention module.

Per batch image b:
  att_enc = encoder_out[b] @ W_enc                      # [P, A]
  c_b     = decoder_hidden[b] @ W_dec + b_enc + b_dec   # [A]
  energy  = relu(att_enc + c_b) @ w_full  (+ b_full)    # [P]
  alpha   = softmax(energy)                             # [P]
  context = alpha @ encoder_out[b]                      # [E]

Data-parallel: batch 512 sharded as 64 per NeuronCore across 8 cores.
b_full cancels inside softmax, and measured energies are in [-1.6, 1.3],
so exp() runs without max subtraction.

Batches are processed in PAIRS with their pixel axes concatenated
(392 = 2*196), which halves matmul/DVE/ACT instruction counts:
  X bf16 (SWDGE cast-DMA) -> TensorE transposes -> X.T pair chunks
  [128e, 392p] -> Y.T = W_enc.T @ X.T (N=392 matmuls, weights
  stationary) -> ScalarE relu+bias per batch half -> energy row
  [1, 392] via w_full-column matmuls -> exp / softmax on the row ->
  exp columns via tiny transposes -> context matmuls col-tiled across
  the pair (PE array columns 0-31 / 32-63 concurrently).
The emit loop is software-pipelined (head(p), energy(p-1), ctx(p-2))
so PE never waits on ScalarE/VectorE evacuations.
"""

from contextlib import ExitStack

import numpy as np

import concourse.bass as bass
import concourse.tile as tile
from concourse import bacc, mybir
from concourse.bass_utils import run_bass_kernel_spmd
from concourse.masks import make_identity

N_CORES = 8
B, P, E, A = 512, 196, 512, 512
BL = B // N_CORES          # 64 batches per core
P0, P1 = 128, P - 128      # 128 + 68 pixel partition tiles
P2 = 2 * P                 # pair-fused pixel axis
EC = E // 128              # 4 contraction chunks
AC = A // 128              # 4 attention-dim chunks

F32 = mybir.dt.float32
BF16 = mybir.dt.bfloat16
AF = mybir.ActivationFunctionType


def build_kernel(BL=BL):
    nc = bacc.Bacc("TRN2", target_bir_lowering=False, debug=False)

    enc = nc.declare_dram_parameter("encoder_out", [BL, P, E], F32, isOutput=False)
    dec = nc.declare_dram_parameter("decoder_hidden", [BL, E], F32, isOutput=False)
    w_enc = nc.declare_dram_parameter("W_enc", [E, A], F32, isOutput=False)
    b_enc = nc.declare_dram_parameter("b_enc", [A], F32, isOutput=False)
    w_dec = nc.declare_dram_parameter("W_dec", [E, A], F32, isOutput=False)
    b_dec = nc.declare_dram_parameter("b_dec", [A], F32, isOutput=False)
    w_full = nc.declare_dram_parameter("w_full", [A], F32, isOutput=False)
    ctx_out = nc.declare_dram_parameter("context", [BL, E], F32, isOutput=True)
    alpha_out = nc.declare_dram_parameter("alpha", [BL, P], F32, isOutput=True)

    # input DMA groups (batches per SWDGE load): small first groups so the
    # PE can start before the bulk of the first loads land
    sizes = []
    left = BL
    for s in [2, 2, 2, 2, 4, 4]:
        if left >= s:
            sizes.append(s)
            left -= s
    while left > 0:
        s = min(8, left)
        sizes.append(s)
        left -= s
    groups = []
    start = 0
    for s in sizes:
        groups.append((start, s))
        start += s
    pair_group = {}
    for gi, (gs, gn) in enumerate(groups):
        for b in range(gs, gs + gn, 2):
            pair_group[b // 2] = (gi, (b - gs))

    with tile.TileContext(nc) as tc, ExitStack() as ctx:
        const = ctx.enter_context(tc.tile_pool(name="const", bufs=1))
        xf_pool = ctx.enter_context(tc.tile_pool(name="xf", bufs=2))
        xg_pool = ctx.enter_context(tc.tile_pool(name="xg", bufs=5))
        xt_pool = ctx.enter_context(tc.tile_pool(name="xt", bufs=3))
        rt_pool = ctx.enter_context(tc.tile_pool(name="rt", bufs=4))
        rows = ctx.enter_context(tc.tile_pool(name="rows", bufs=4))
        tp_ps = ctx.enter_context(tc.tile_pool(name="tp_ps", bufs=2, space="PSUM"))
        yp_ps = ctx.enter_context(tc.tile_pool(name="yp_ps", bufs=3, space="PSUM"))
        ep_ps = ctx.enter_context(tc.tile_pool(name="ep_ps", bufs=1, space="PSUM"))
        cp_ps = ctx.enter_context(tc.tile_pool(name="cp_ps", bufs=2, space="PSUM"))

        # ---- constants ----
        state = {}

        def emit_group_dma(gi):
            gs, gn = groups[gi]
            xg = xg_pool.tile([128, 8, 2, E], BF16, tag="xg")
            if gi < 2:
                # ramp groups ride the HWDGE/sync queue (in parallel with
                # the SWDGE weight loads) as f32 + ScalarE convert; the
                # head transposes read the f32 tile directly so only the
                # context path (2 pairs later) waits on the convert
                xf = xf_pool.tile([128, 2, 2, E], F32, tag="xf")
                nc.sync.dma_start(
                    out=xf[:, 0:gn, 0, :],
                    in_=enc[gs:gs + gn, 0:P0, :].rearrange("b p e -> p b e"))
                nc.sync.dma_start(
                    out=xf[0:P1, 0:gn, 1, :],
                    in_=enc[gs:gs + gn, P0:P, :].rearrange("b p e -> p b e"))
                nc.scalar.copy(xg[:, 0:gn, 0, :], xf[:, 0:gn, 0, :])
                nc.scalar.copy(xg[0:P1, 0:gn, 1, :], xf[0:P1, 0:gn, 1, :])
                state[("xf", gi)] = xf
            else:
                nc.gpsimd.dma_start(
                    out=xg[:, 0:gn, 0, :],
                    in_=enc[gs:gs + gn, 0:P0, :].rearrange("b p e -> p b e"))
                nc.gpsimd.dma_start(
                    out=xg[0:P1, 0:gn, 1, :],
                    in_=enc[gs:gs + gn, P0:P, :].rearrange("b p e -> p b e"))
            state[("g", gi)] = xg

        # Startup-critical SWDGE queue order (single FIFO per engine):
        # group 0's X, then W_enc chunk 0 (first main matmuls),
        # decoder_hidden + W_dec (the C.T preamble feeding the first
        # relus), group 1, remaining W_enc chunks, ...  All weights load
        # as bf16 via cast-DMA so no on-chip converts gate anything.
        wenc_bf = const.tile([128, EC, A], BF16)   # [e_in_chunk, e_chunk, a]
        wdec_bf = const.tile([128, EC, A], BF16)
        wf_bf = const.tile([128, AC], BF16)        # w_full as columns per a-chunk
        h_bf = const.tile([BL, E], BF16)

        ident = const.tile([128, 128], BF16)
        make_identity(nc, ident)
        identf = const.tile([128, 128], F32)
        make_identity(nc, identf)

        emit_group_dma(0)
        nc.gpsimd.dma_start(out=h_bf, in_=dec[:, :])
        nc.gpsimd.dma_start(out=wenc_bf[:, 0, :], in_=w_enc[0:128, :])
        for c in range(EC):
            nc.gpsimd.dma_start(out=wdec_bf[:, c, :],
                                in_=w_dec[c * 128:(c + 1) * 128, :])
        if len(groups) > 1:
            emit_group_dma(1)
        for c in range(1, EC):
            nc.gpsimd.dma_start(out=wenc_bf[:, c, :],
                                in_=w_enc[c * 128:(c + 1) * 128, :])
        nc.gpsimd.dma_start(out=wf_bf, in_=w_full.ap().rearrange("(c p) -> p c", p=128))
        if len(groups) > 2:
            emit_group_dma(2)

        bias_e = const.tile([128, AC], F32)
        bias_d = const.tile([128, AC], F32)
        bias_c = const.tile([128, AC], F32)
        nc.sync.dma_start(out=bias_e, in_=b_enc.ap().rearrange("(c p) -> p c", p=128))
        nc.sync.dma_start(out=bias_d, in_=b_dec.ap().rearrange("(c p) -> p c", p=128))
        nc.vector.tensor_add(bias_c, bias_e, bias_d)

        # HAM warm-up: the PE is otherwise idle while the first X group and
        # W_dec load, and the clock gate needs ~3.4us of matmul activity to
        # reach 2.4 GHz; burn the wait on dummy matmuls (they only depend on
        # the identity tile) so the first real pairs run warm
        warm = ep_ps.tile([128, 512], F32, tag="ep")
        for _ in range(28):
            nc.tensor.matmul(warm[:, 0:128], ident, ident, start=True, stop=True)

        # H.T [e, b] via TensorE transposes
        ht_bf = const.tile([128, EC, BL], BF16)
        for c in range(EC):
            tp = tp_ps.tile([128, P2], BF16, tag="tp")
            nc.tensor.transpose(tp[:, 0:BL], h_bf[0:BL, c * 128:(c + 1) * 128],
                                ident[0:BL, 0:BL])
            nc.vector.tensor_copy(ht_bf[:, c, :], tp[:, 0:BL])

        # C.T[a, b] = W_dec.T @ H.T + (b_enc + b_dec), per-partition bias add
        ct = const.tile([128, AC, BL], F32)
        for ac in range(AC):
            cps = tp_ps.tile([128, P2], F32, tag="tp")
            for ec in range(EC):
                nc.tensor.matmul(cps[:, 0:BL], wdec_bf[:, ec, ac * 128:(ac + 1) * 128],
                                 ht_bf[:, ec, :], start=(ec == 0), stop=(ec == EC - 1))
            nc.scalar.activation(ct[:, ac, :], cps[:, 0:BL], AF.Identity,
                                 bias=bias_c[:, ac:ac + 1])

        # ---- pair loop, software-pipelined ----
        def emit_head(pbs):
            # one or two pairs; with two, the main matmuls run ec-major
            # across both pairs so consecutive matmuls share the stationary
            # W_enc tile (bacc elides the duplicate LDWEIGHTS)
            for pb in pbs:
                gi, off = pair_group[pb]
                if ("g", gi) not in state:
                    emit_group_dma(gi)
                xg = state[("g", gi)]
                x0a = xg[:, off, 0, :]           # batch b   pixels 0:128
                x1a = xg[0:P1, off, 1, :]        # batch b   pixels 128:196
                x0b = xg[:, off + 1, 0, :]       # batch b+1 pixels 0:128
                x1b = xg[0:P1, off + 1, 1, :]    # batch b+1 pixels 128:196

                # X.T pair chunks [128e, 392p] = [b0:196 | b1:196]; ramp
                # groups transpose from the f32 tile (fp32 transpose-mode
                # is full rate, copy does the bf16 cast)
                if ("xf", gi) in state:
                    xf = state[("xf", gi)]
                    t0a, t1a = xf[:, off, 0, :], xf[0:P1, off, 1, :]
                    t0b, t1b = xf[:, off + 1, 0, :], xf[0:P1, off + 1, 1, :]
                    tid, tdt = identf, F32
                else:
                    t0a, t1a, t0b, t1b = x0a, x1a, x0b, x1b
                    tid, tdt = ident, BF16
                xt = xt_pool.tile([128, EC, P2], BF16, tag="xt")
                for ec in range(EC):
                    sl = slice(ec * 128, (ec + 1) * 128)
                    tp = tp_ps.tile([128, P2], tdt, tag="tp")
                    nc.tensor.transpose(tp[:, 0:P0], t0a[:, sl], tid)
                    nc.tensor.transpose(tp[:, P0:P], t1a[:, sl], tid[0:P1, 0:P1])
                    nc.tensor.transpose(tp[:, P:P + P0], t0b[:, sl], tid)
                    nc.tensor.transpose(tp[:, P + P0:P2], t1b[:, sl], tid[0:P1, 0:P1])
                    nc.vector.tensor_copy(xt[:, ec, :], tp)
                state[pb] = dict(x0a=x0a, x1a=x1a, x0b=x0b, x1b=x1b, xt=xt)

            # Y.T pair chunks + fused relu/bias/cast per batch half
            for pb in pbs:
                b = 2 * pb
                rt = rt_pool.tile([128, AC, P2], BF16, tag="rt")
                state[pb]["rt"] = rt
                for ac in range(AC):
                    yp = yp_ps.tile([128, P2], F32, tag="yp")
                    for ec in range(EC):
                        nc.tensor.matmul(yp, wenc_bf[:, ec, ac * 128:(ac + 1) * 128],
                                         state[pb]["xt"][:, ec, :],
                                         start=(ec == 0), stop=(ec == EC - 1))
                    nc.scalar.activation(rt[:, ac, 0:P], yp[:, 0:P], AF.Relu,
                                         bias=ct[:, ac, b:b + 1])
                    nc.scalar.activation(rt[:, ac, P:P2], yp[:, P:P2], AF.Relu,
                                         bias=ct[:, ac, b + 1:b + 2])

        def emit_tail(pe_pair, cx_pair):
            if pe_pair is not None:
                # energy matmuls + softmax tail (no max subtraction;
                # energies are O(1))
                b = 2 * pe_pair
                st = state[pe_pair]
                rt = st["rt"]
                ept = ep_ps.tile([64, 512], F32, tag="ep")
                ep = ept[0:1, 0:P2]
                for ac in range(AC):
                    nc.tensor.matmul(ep, wf_bf[:, ac:ac + 1], rt[:, ac, :],
                                     start=(ac == 0), stop=(ac == AC - 1),
                                     tile_position=(0, 0))
                exp_bf = rows.tile([1, P2], BF16, tag="exp")
                zsum = rows.tile([1, 2], F32, tag="zsum")
                rz = rows.tile([1, 2], F32, tag="rz")
                nc.scalar.activation(exp_bf, ep, AF.Exp)
                nc.vector.tensor_reduce(zsum, exp_bf.rearrange("r (b p) -> r b p", b=2),
                                        axis=mybir.AxisListType.X, op=mybir.AluOpType.add)
                nc.vector.reciprocal(rz, zsum)
                alpha_row = rows.tile([1, P2], F32, tag="alpha")
                nc.scalar.mul(alpha_row[:, 0:P], exp_bf[:, 0:P], rz[:, 0:1])
                nc.scalar.mul(alpha_row[:, P:P2], exp_bf[:, P:P2], rz[:, 1:2])
                nc.sync.dma_start(out=alpha_out[b:b + 2, :], in_=alpha_row)
                st.update(exp_bf=exp_bf, rz=rz)

            if cx_pair is not None:
                b2 = 2 * cx_pair
                st2 = state.pop(cx_pair)
                exp2, rz2 = st2["exp_bf"], st2["rz"]
                idk = ident[0:1, 0:1]
                # exp columns: psum cols {0,2} = batch b p0/p1, {4,6} = b+1
                ecol_p = tp_ps.tile([128, 8], BF16, tag="tp")
                nc.tensor.transpose(ecol_p[:, 0:1], exp2[0:1, 0:P0], idk)
                nc.tensor.transpose(ecol_p[0:P1, 2:3], exp2[0:1, P0:P], idk)
                nc.tensor.transpose(ecol_p[:, 4:5], exp2[0:1, P:P + P0], idk)
                nc.tensor.transpose(ecol_p[0:P1, 6:7], exp2[0:1, P + P0:P2], idk)
                ecol = rows.tile([128, 4], BF16, tag="ecolsb")
                nc.vector.tensor_copy(ecol[:, 0:2], ecol_p[:, 0:5:4])
                nc.vector.tensor_copy(ecol[0:P1, 2:4], ecol_p[0:P1, 2:7:4])
                # context matmuls, col-tiled across the pair (cols 0-31/32-63)
                cpa = cp_ps.tile([64, 512], F32, tag="cp")
                cpb = cp_ps.tile([64, 512], F32, tag="cp")
                nc.tensor.matmul(cpa[0:1, :], ecol[:, 0:1], st2["x0a"],
                                 start=True, stop=False, tile_position=(0, 0))
                nc.tensor.matmul(cpb[32:33, :], ecol[:, 1:2], st2["x0b"],
                                 start=True, stop=False, tile_position=(0, 32))
                nc.tensor.matmul(cpa[0:1, :], ecol[0:P1, 2:3], st2["x1a"],
                                 start=False, stop=True, tile_position=(0, 0))
                nc.tensor.matmul(cpb[32:33, :], ecol[0:P1, 3:4], st2["x1b"],
                                 start=False, stop=True, tile_position=(0, 32))
                ctx2 = rows.tile([33, E], F32, tag="ctx")
                nc.vector.tensor_scalar_mul(ctx2[0:1, :], cpa[0:1, :], rz2[:, 0:1])
                nc.vector.tensor_scalar_mul(ctx2[32:33, :], cpb[32:33, :], rz2[:, 1:2])
                nc.sync.dma_start(out=ctx_out[b2:b2 + 2, :], in_=ctx2[0:33:32, :])

        NP = BL // 2
        for pb in range(NP):
            emit_head([pb])
            emit_tail(pb - 1 if pb >= 1 else None, pb - 2 if pb >= 2 else None)
        emit_tail(NP - 1, NP - 2 if NP >= 2 else None)
        emit_tail(None, NP - 1)

    nc.compile()
    return nc


_NC = None


def kernel(**inputs):
    global _NC
    if _NC is None:
        _NC = build_kernel()
    nc = _NC

    enc = np.ascontiguousarray(inputs["encoder_out"], dtype=np.float32)
    dec = np.ascontiguousarray(inputs["decoder_hidden"], dtype=np.float32)
    shared = {
        "W_enc": np.ascontiguousarray(inputs["W_enc"], dtype=np.float32),
        "b_enc": np.ascontiguousarray(inputs["b_enc"], dtype=np.float32),
        "W_dec": np.ascontiguousarray(inputs["W_dec"], dtype=np.float32),
        "b_dec": np.ascontiguousarray(inputs["b_dec"], dtype=np.float32),
        "w_full": np.ascontiguousarray(inputs["w_full"], dtype=np.float32),
    }
    in_maps = []
    for i in range(N_CORES):
        m = dict(shared)
        m["encoder_out"] = enc[i * BL:(i + 1) * BL]
        m["decoder_hidden"] = dec[i * BL:(i + 1) * BL]
        in_maps.append(m)

    res = run_bass_kernel_spmd(nc, in_maps, list(range(N_CORES)))
    context = np.concatenate([res.results[i]["context"] for i in range(N_CORES)], axis=0)
    alpha = np.concatenate([res.results[i]["alpha"] for i in range(N_CORES)], axis=0)
    return context.astype(np.float32), alpha.astype(np.float32)


# revision 67
# speedup vs baseline: 1.0182x; 1.0003x over previous
"""Trainium2 Bass kernel for the show-attend-tell style attention module.

Per batch image b:
  att_enc = encoder_out[b] @ W_enc                      # [P, A]
  c_b     = decoder_hidden[b] @ W_dec + b_enc + b_dec   # [A]
  energy  = relu(att_enc + c_b) @ w_full  (+ b_full)    # [P]
  alpha   = softmax(energy)                             # [P]
  context = alpha @ encoder_out[b]                      # [E]

Data-parallel: batch 512 sharded as 64 per NeuronCore across 8 cores.
b_full cancels inside softmax, and measured energies are in [-1.6, 1.3],
so exp() runs without max subtraction.

Batches are processed in PAIRS with their pixel axes concatenated
(392 = 2*196), which halves matmul/DVE/ACT instruction counts:
  X bf16 (SWDGE cast-DMA) -> TensorE transposes -> X.T pair chunks
  [128e, 392p] -> Y.T = W_enc.T @ X.T (N=392 matmuls, weights
  stationary) -> ScalarE relu+bias per batch half -> energy row
  [1, 392] via w_full-column matmuls -> exp / softmax on the row ->
  exp columns via tiny transposes -> context matmuls col-tiled across
  the pair (PE array columns 0-31 / 32-63 concurrently).
The emit loop is software-pipelined (head(p), energy(p-1), ctx(p-2))
so PE never waits on ScalarE/VectorE evacuations.
"""

from contextlib import ExitStack

import numpy as np

import concourse.bass as bass
import concourse.tile as tile
from concourse import bacc, mybir
from concourse.bass_utils import run_bass_kernel_spmd
from concourse.masks import make_identity

N_CORES = 8
B, P, E, A = 512, 196, 512, 512
BL = B // N_CORES          # 64 batches per core
P0, P1 = 128, P - 128      # 128 + 68 pixel partition tiles
P2 = 2 * P                 # pair-fused pixel axis
EC = E // 128              # 4 contraction chunks
AC = A // 128              # 4 attention-dim chunks

F32 = mybir.dt.float32
BF16 = mybir.dt.bfloat16
AF = mybir.ActivationFunctionType


def build_kernel(BL=BL):
    nc = bacc.Bacc("TRN2", target_bir_lowering=False, debug=False)

    enc = nc.declare_dram_parameter("encoder_out", [BL, P, E], F32, isOutput=False)
    dec = nc.declare_dram_parameter("decoder_hidden", [BL, E], F32, isOutput=False)
    w_enc = nc.declare_dram_parameter("W_enc", [E, A], F32, isOutput=False)
    b_enc = nc.declare_dram_parameter("b_enc", [A], F32, isOutput=False)
    w_dec = nc.declare_dram_parameter("W_dec", [E, A], F32, isOutput=False)
    b_dec = nc.declare_dram_parameter("b_dec", [A], F32, isOutput=False)
    w_full = nc.declare_dram_parameter("w_full", [A], F32, isOutput=False)
    ctx_out = nc.declare_dram_parameter("context", [BL, E], F32, isOutput=True)
    alpha_out = nc.declare_dram_parameter("alpha", [BL, P], F32, isOutput=True)

    # input DMA groups (batches per SWDGE load): small first groups so the
    # PE can start before the bulk of the first loads land
    sizes = []
    left = BL
    for s in [2, 2, 2, 2, 4, 4]:
        if left >= s:
            sizes.append(s)
            left -= s
    while left > 0:
        s = min(8, left)
        sizes.append(s)
        left -= s
    groups = []
    start = 0
    for s in sizes:
        groups.append((start, s))
        start += s
    pair_group = {}
    for gi, (gs, gn) in enumerate(groups):
        for b in range(gs, gs + gn, 2):
            pair_group[b // 2] = (gi, (b - gs))

    with tile.TileContext(nc) as tc, ExitStack() as ctx:
        const = ctx.enter_context(tc.tile_pool(name="const", bufs=1))
        xf_pool = ctx.enter_context(tc.tile_pool(name="xf", bufs=2))
        xg_pool = ctx.enter_context(tc.tile_pool(name="xg", bufs=5))
        xt_pool = ctx.enter_context(tc.tile_pool(name="xt", bufs=3))
        rt_pool = ctx.enter_context(tc.tile_pool(name="rt", bufs=4))
        rows = ctx.enter_context(tc.tile_pool(name="rows", bufs=4))
        tp_ps = ctx.enter_context(tc.tile_pool(name="tp_ps", bufs=2, space="PSUM"))
        yp_ps = ctx.enter_context(tc.tile_pool(name="yp_ps", bufs=3, space="PSUM"))
        ep_ps = ctx.enter_context(tc.tile_pool(name="ep_ps", bufs=1, space="PSUM"))
        cp_ps = ctx.enter_context(tc.tile_pool(name="cp_ps", bufs=2, space="PSUM"))

        # ---- constants ----
        state = {}

        def emit_group_dma(gi):
            gs, gn = groups[gi]
            xg = xg_pool.tile([128, 8, 2, E], BF16, tag="xg")
            if gi < 2:
                # ramp groups ride the HWDGE/sync queue (in parallel with
                # the SWDGE weight loads) as f32 + ScalarE convert; the
                # head transposes read the f32 tile directly so only the
                # context path (2 pairs later) waits on the convert
                xf = xf_pool.tile([128, 2, 2, E], F32, tag="xf")
                nc.sync.dma_start(
                    out=xf[:, 0:gn, 0, :],
                    in_=enc[gs:gs + gn, 0:P0, :].rearrange("b p e -> p b e"))
                nc.sync.dma_start(
                    out=xf[0:P1, 0:gn, 1, :],
                    in_=enc[gs:gs + gn, P0:P, :].rearrange("b p e -> p b e"))
                nc.scalar.copy(xg[:, 0:gn, 0, :], xf[:, 0:gn, 0, :])
                nc.scalar.copy(xg[0:P1, 0:gn, 1, :], xf[0:P1, 0:gn, 1, :])
                state[("xf", gi)] = xf
            else:
                nc.gpsimd.dma_start(
                    out=xg[:, 0:gn, 0, :],
                    in_=enc[gs:gs + gn, 0:P0, :].rearrange("b p e -> p b e"))
                nc.gpsimd.dma_start(
                    out=xg[0:P1, 0:gn, 1, :],
                    in_=enc[gs:gs + gn, P0:P, :].rearrange("b p e -> p b e"))
            state[("g", gi)] = xg

        # Startup-critical SWDGE queue order (single FIFO per engine):
        # group 0's X, then W_enc chunk 0 (first main matmuls),
        # decoder_hidden + W_dec (the C.T preamble feeding the first
        # relus), group 1, remaining W_enc chunks, ...  All weights load
        # as bf16 via cast-DMA so no on-chip converts gate anything.
        wenc_bf = const.tile([128, EC, A], BF16)   # [e_in_chunk, e_chunk, a]
        wdec_bf = const.tile([128, EC, A], BF16)
        wf_bf = const.tile([128, AC], BF16)        # w_full as columns per a-chunk
        h_bf = const.tile([BL, E], BF16)

        ident = const.tile([128, 128], BF16)
        make_identity(nc, ident)
        identf = const.tile([128, 128], F32)
        make_identity(nc, identf)

        # tiny bias loads go first on the sync queue so the DVE-FIFO-head
        # bias_add clears immediately instead of gating the X.T copies
        bias_e = const.tile([128, AC], F32)
        bias_d = const.tile([128, AC], F32)
        bias_c = const.tile([128, AC], F32)
        nc.sync.dma_start(out=bias_e, in_=b_enc.ap().rearrange("(c p) -> p c", p=128))
        nc.sync.dma_start(out=bias_d, in_=b_dec.ap().rearrange("(c p) -> p c", p=128))
        nc.vector.tensor_add(bias_c, bias_e, bias_d)

        emit_group_dma(0)
        nc.gpsimd.dma_start(out=h_bf, in_=dec[:, :])
        nc.gpsimd.dma_start(out=wenc_bf[:, 0, :], in_=w_enc[0:128, :])
        for c in range(EC):
            nc.gpsimd.dma_start(out=wdec_bf[:, c, :],
                                in_=w_dec[c * 128:(c + 1) * 128, :])
        if len(groups) > 1:
            emit_group_dma(1)
        for c in range(1, EC):
            nc.gpsimd.dma_start(out=wenc_bf[:, c, :],
                                in_=w_enc[c * 128:(c + 1) * 128, :])
        nc.gpsimd.dma_start(out=wf_bf, in_=w_full.ap().rearrange("(c p) -> p c", p=128))
        if len(groups) > 2:
            emit_group_dma(2)

        # HAM warm-up: the PE is otherwise idle while the first X group and
        # W_dec load, and the clock gate needs ~3.4us of matmul activity to
        # reach 2.4 GHz; burn the wait on dummy matmuls (they only depend on
        # the identity tile) so the first real pairs run warm
        warm = ep_ps.tile([128, 512], F32, tag="ep")
        for _ in range(28):
            nc.tensor.matmul(warm[:, 0:128], ident, ident, start=True, stop=True)

        # H.T [e, b] via TensorE transposes
        ht_bf = const.tile([128, EC, BL], BF16)
        for c in range(EC):
            tp = tp_ps.tile([128, P2], BF16, tag="tp")
            nc.tensor.transpose(tp[:, 0:BL], h_bf[0:BL, c * 128:(c + 1) * 128],
                                ident[0:BL, 0:BL])
            nc.vector.tensor_copy(ht_bf[:, c, :], tp[:, 0:BL])

        # C.T[a, b] = W_dec.T @ H.T + (b_enc + b_dec), per-partition bias add
        ct = const.tile([128, AC, BL], F32)
        for ac in range(AC):
            cps = tp_ps.tile([128, P2], F32, tag="tp")
            for ec in range(EC):
                nc.tensor.matmul(cps[:, 0:BL], wdec_bf[:, ec, ac * 128:(ac + 1) * 128],
                                 ht_bf[:, ec, :], start=(ec == 0), stop=(ec == EC - 1))
            nc.scalar.activation(ct[:, ac, :], cps[:, 0:BL], AF.Identity,
                                 bias=bias_c[:, ac:ac + 1])

        # ---- pair loop, software-pipelined ----
        def emit_head(pbs):
            # one or two pairs; with two, the main matmuls run ec-major
            # across both pairs so consecutive matmuls share the stationary
            # W_enc tile (bacc elides the duplicate LDWEIGHTS)
            for pb in pbs:
                gi, off = pair_group[pb]
                if ("g", gi) not in state:
                    emit_group_dma(gi)
                xg = state[("g", gi)]
                x0a = xg[:, off, 0, :]           # batch b   pixels 0:128
                x1a = xg[0:P1, off, 1, :]        # batch b   pixels 128:196
                x0b = xg[:, off + 1, 0, :]       # batch b+1 pixels 0:128
                x1b = xg[0:P1, off + 1, 1, :]    # batch b+1 pixels 128:196

                # X.T pair chunks [128e, 392p] = [b0:196 | b1:196]; ramp
                # groups transpose from the f32 tile (fp32 transpose-mode
                # is full rate, copy does the bf16 cast)
                if ("xf", gi) in state:
                    xf = state[("xf", gi)]
                    t0a, t1a = xf[:, off, 0, :], xf[0:P1, off, 1, :]
                    t0b, t1b = xf[:, off + 1, 0, :], xf[0:P1, off + 1, 1, :]
                    tid, tdt = identf, F32
                else:
                    t0a, t1a, t0b, t1b = x0a, x1a, x0b, x1b
                    tid, tdt = ident, BF16
                xt = xt_pool.tile([128, EC, P2], BF16, tag="xt")
                for ec in range(EC):
                    sl = slice(ec * 128, (ec + 1) * 128)
                    tp = tp_ps.tile([128, P2], tdt, tag="tp")
                    nc.tensor.transpose(tp[:, 0:P0], t0a[:, sl], tid)
                    nc.tensor.transpose(tp[:, P0:P], t1a[:, sl], tid[0:P1, 0:P1])
                    nc.tensor.transpose(tp[:, P:P + P0], t0b[:, sl], tid)
                    nc.tensor.transpose(tp[:, P + P0:P2], t1b[:, sl], tid[0:P1, 0:P1])
                    nc.vector.tensor_copy(xt[:, ec, :], tp)
                state[pb] = dict(x0a=x0a, x1a=x1a, x0b=x0b, x1b=x1b, xt=xt)

            # Y.T pair chunks + fused relu/bias/cast per batch half
            for pb in pbs:
                b = 2 * pb
                rt = rt_pool.tile([128, AC, P2], BF16, tag="rt")
                state[pb]["rt"] = rt
                for ac in range(AC):
                    yp = yp_ps.tile([128, P2], F32, tag="yp")
                    for ec in range(EC):
                        nc.tensor.matmul(yp, wenc_bf[:, ec, ac * 128:(ac + 1) * 128],
                                         state[pb]["xt"][:, ec, :],
                                         start=(ec == 0), stop=(ec == EC - 1))
                    nc.scalar.activation(rt[:, ac, 0:P], yp[:, 0:P], AF.Relu,
                                         bias=ct[:, ac, b:b + 1])
                    nc.scalar.activation(rt[:, ac, P:P2], yp[:, P:P2], AF.Relu,
                                         bias=ct[:, ac, b + 1:b + 2])

        def emit_tail(pe_pair, cx_pair):
            if pe_pair is not None:
                # energy matmuls + softmax tail (no max subtraction;
                # energies are O(1))
                b = 2 * pe_pair
                st = state[pe_pair]
                rt = st["rt"]
                ept = ep_ps.tile([64, 512], F32, tag="ep")
                ep = ept[0:1, 0:P2]
                for ac in range(AC):
                    nc.tensor.matmul(ep, wf_bf[:, ac:ac + 1], rt[:, ac, :],
                                     start=(ac == 0), stop=(ac == AC - 1),
                                     tile_position=(0, 0))
                exp_bf = rows.tile([1, P2], BF16, tag="exp")
                zsum = rows.tile([1, 2], F32, tag="zsum")
                rz = rows.tile([1, 2], F32, tag="rz")
                nc.scalar.activation(exp_bf, ep, AF.Exp)
                nc.vector.tensor_reduce(zsum, exp_bf.rearrange("r (b p) -> r b p", b=2),
                                        axis=mybir.AxisListType.X, op=mybir.AluOpType.add)
                nc.vector.reciprocal(rz, zsum)
                alpha_row = rows.tile([1, P2], F32, tag="alpha")
                nc.scalar.mul(alpha_row[:, 0:P], exp_bf[:, 0:P], rz[:, 0:1])
                nc.scalar.mul(alpha_row[:, P:P2], exp_bf[:, P:P2], rz[:, 1:2])
                nc.sync.dma_start(out=alpha_out[b:b + 2, :], in_=alpha_row)
                st.update(exp_bf=exp_bf, rz=rz)

            if cx_pair is not None:
                b2 = 2 * cx_pair
                st2 = state.pop(cx_pair)
                exp2, rz2 = st2["exp_bf"], st2["rz"]
                idk = ident[0:1, 0:1]
                # exp columns: psum cols {0,2} = batch b p0/p1, {4,6} = b+1
                ecol_p = tp_ps.tile([128, 8], BF16, tag="tp")
                nc.tensor.transpose(ecol_p[:, 0:1], exp2[0:1, 0:P0], idk)
                nc.tensor.transpose(ecol_p[0:P1, 2:3], exp2[0:1, P0:P], idk)
                nc.tensor.transpose(ecol_p[:, 4:5], exp2[0:1, P:P + P0], idk)
                nc.tensor.transpose(ecol_p[0:P1, 6:7], exp2[0:1, P + P0:P2], idk)
                ecol = rows.tile([128, 4], BF16, tag="ecolsb")
                nc.vector.tensor_copy(ecol[:, 0:2], ecol_p[:, 0:5:4])
                nc.vector.tensor_copy(ecol[0:P1, 2:4], ecol_p[0:P1, 2:7:4])
                # context matmuls, col-tiled across the pair (cols 0-31/32-63)
                cpa = cp_ps.tile([64, 512], F32, tag="cp")
                cpb = cp_ps.tile([64, 512], F32, tag="cp")
                nc.tensor.matmul(cpa[0:1, :], ecol[:, 0:1], st2["x0a"],
                                 start=True, stop=False, tile_position=(0, 0))
                nc.tensor.matmul(cpb[32:33, :], ecol[:, 1:2], st2["x0b"],
                                 start=True, stop=False, tile_position=(0, 32))
                nc.tensor.matmul(cpa[0:1, :], ecol[0:P1, 2:3], st2["x1a"],
                                 start=False, stop=True, tile_position=(0, 0))
                nc.tensor.matmul(cpb[32:33, :], ecol[0:P1, 3:4], st2["x1b"],
                                 start=False, stop=True, tile_position=(0, 32))
                ctx2 = rows.tile([33, E], F32, tag="ctx")
                nc.vector.tensor_scalar_mul(ctx2[0:1, :], cpa[0:1, :], rz2[:, 0:1])
                nc.vector.tensor_scalar_mul(ctx2[32:33, :], cpb[32:33, :], rz2[:, 1:2])
                nc.sync.dma_start(out=ctx_out[b2:b2 + 2, :], in_=ctx2[0:33:32, :])

        NP = BL // 2
        for pb in range(NP):
            emit_head([pb])
            emit_tail(pb - 1 if pb >= 1 else None, pb - 2 if pb >= 2 else None)
        emit_tail(NP - 1, NP - 2 if NP >= 2 else None)
        emit_tail(None, NP - 1)

    nc.compile()
    return nc


_NC = None


def kernel(**inputs):
    global _NC
    if _NC is None:
        _NC = build_kernel()
    nc = _NC

    enc = np.ascontiguousarray(inputs["encoder_out"], dtype=np.float32)
    dec = np.ascontiguousarray(inputs["decoder_hidden"], dtype=np.float32)
    shared = {
        "W_enc": np.ascontiguousarray(inputs["W_enc"], dtype=np.float32),
        "b_enc": np.ascontiguousarray(inputs["b_enc"], dtype=np.float32),
        "W_dec": np.ascontiguousarray(inputs["W_dec"], dtype=np.float32),
        "b_dec": np.ascontiguousarray(inputs["b_dec"], dtype=np.float32),
        "w_full": np.ascontiguousarray(inputs["w_full"], dtype=np.float32),
    }
    in_maps = []
    for i in range(N_CORES):
        m = dict(shared)
        m["encoder_out"] = enc[i * BL:(i + 1) * BL]
        m["decoder_hidden"] = dec[i * BL:(i + 1) * BL]
        in_maps.append(m)

    res = run_bass_kernel_spmd(nc, in_maps, list(range(N_CORES)))
    context = np.concatenate([res.results[i]["context"] for i in range(N_CORES)], axis=0)
    alpha = np.concatenate([res.results[i]["alpha"] for i in range(N_CORES)], axis=0)
    return context.astype(np.float32), alpha.astype(np.float32)
